# revision 7
# baseline (speedup 1.0000x reference)
"""Expert-parallel MoE MLP kernel for TRN2 (8 NeuronCores, 1 expert/core).

Math per core (expert e):
    h   = gelu(x_e @ w1_e + b1_e)      x_e: [4096, 1024], w1_e: [1024, 4096]
    out = h @ w2_e + b2_e              w2_e: [4096, 1024]

Layout strategy:
  - Contraction dims must live on SBUF partitions, so mm1 computes hT
    ([H, tok]) with stationary = w1 tile, moving = xT tile; mm2 then uses
    hT tiles directly as the stationary operand and produces out in
    natural [tok, D] layout.
  - x is transposed 128x128-blockwise on the PE (fp32, identity-matmul
    transpose); the PSUM->SBUF copy converts to bf16.
  - All matmuls run in bf16 (1 cycle/row on the PE vs 4 for fp32);
    accumulation is fp32 in PSUM. GELU runs on ACT from PSUM with the
    exact-erf table, adding b1 as the per-partition activation bias.
"""

import numpy as np

import concourse.bacc as bacc
import concourse.bass as bass
import concourse.mybir as mybir
import concourse.tile as tile
from concourse import bass_utils
from concourse.masks import make_identity

P = 128
D = 1024
H = 4096
NTOK = 4096  # B*N per expert
NCORES = 8
CHUNK = 512  # tokens per pipeline chunk
NCHUNK = NTOK // CHUNK
F32 = mybir.dt.float32
BF16 = mybir.dt.bfloat16
GELU = mybir.ActivationFunctionType.Gelu

DK = D // P  # 8   k-tiles of D
HM = H // P  # 32  tiles of H
TSUB = CHUNK // P  # 4 token subtiles per chunk
DC = D // 512  # 2  512-wide output column chunks


def build_program(act=GELU):
    nc = bacc.Bacc("TRN2", target_bir_lowering=False, debug=False,
                   num_devices=NCORES)

    x = nc.dram_tensor("x", (NTOK, D), F32, kind="ExternalInput").ap()
    w1 = nc.dram_tensor("w1", (D, H), F32, kind="ExternalInput").ap()
    b1 = nc.dram_tensor("b1", (H,), F32, kind="ExternalInput").ap()
    w2 = nc.dram_tensor("w2", (H, D), F32, kind="ExternalInput").ap()
    b2 = nc.dram_tensor("b2", (D,), F32, kind="ExternalInput").ap()
    out = nc.dram_tensor("out", (NTOK, D), F32, kind="ExternalOutput").ap()

    with tile.TileContext(nc) as tc:
        with (
            tc.tile_pool(name="consts", bufs=1) as consts,
            tc.tile_pool(name="weights", bufs=1) as wpool,
            tc.tile_pool(name="stage", bufs=2) as stage,
            tc.tile_pool(name="xt", bufs=1) as xtp,
            tc.tile_pool(name="ht", bufs=1) as htp,
            tc.tile_pool(name="outp", bufs=3) as outp,
            tc.tile_pool(name="pt", bufs=2, space="PSUM") as ptp,
            tc.tile_pool(name="p1", bufs=2, space="PSUM") as p1p,
            tc.tile_pool(name="p2", bufs=4, space="PSUM") as p2p,
        ):
            # ---- constants ----
            ident = consts.tile([P, P], F32, tag="ident")
            make_identity(nc, ident)

            # b1 with H on partitions: [128, 32]; col hm serves psum1 tile hm
            b1_sb = consts.tile([P, HM], F32, tag="b1")
            nc.sync.dma_start(b1_sb, b1.rearrange("(o p) -> p o", p=P))

            # b2 replicated across partitions for the DVE add on out tiles
            b2_row = consts.tile([1, D], F32, tag="b2row")
            nc.sync.dma_start(b2_row, b2[None, :])
            b2_rep = consts.tile([P, D], F32, tag="b2rep")
            nc.gpsimd.partition_broadcast(b2_rep, b2_row)

            # ---- weights: fp32 DRAM -> bf16 SBUF resident ----
            w1_sb = wpool.tile([P, DK, H], BF16, tag="w1")  # [128, 8, 4096]
            for dk in range(DK):
                for hf in range(2):
                    st = stage.tile([P, H // 2], F32, tag="stage")
                    nc.sync.dma_start(
                        st, w1[dk * P:(dk + 1) * P,
                               hf * (H // 2):(hf + 1) * (H // 2)])
                    nc.vector.tensor_copy(
                        w1_sb[:, dk, hf * (H // 2):(hf + 1) * (H // 2)], st)

            w2_sb = wpool.tile([P, HM, D], BF16, tag="w2")  # [128, 32, 1024]
            for g in range(HM // 2):
                st = stage.tile([P, 2, D], F32, tag="stage")
                nc.sync.dma_start(
                    st, w2[g * 2 * P:(g + 1) * 2 * P, :].rearrange(
                        "(o p) d -> p o d", p=P))
                nc.vector.tensor_copy(w2_sb[:, g * 2:(g + 1) * 2, :], st)

            # ---- main pipeline over token chunks ----
            for c in range(NCHUNK):
                # transpose x chunk -> xT tiles (bf16, D on partitions)
                xt = [xtp.tile([P, CHUNK], BF16, tag=f"xt{dk}",
                               name=f"xt{dk}_c{c}") for dk in range(DK)]
                for half in range(2):
                    r0 = c * CHUNK + half * (CHUNK // 2)
                    xs = stage.tile([P, 2, D], F32, tag="stage")
                    nc.sync.dma_start(
                        xs, x[r0:r0 + CHUNK // 2, :].rearrange(
                            "(i p) d -> p i d", p=P))
                    for i in range(2):
                        ts = half * 2 + i
                        for dk in range(DK):
                            pt = ptp.tile([P, P], F32, tag="pt")
                            nc.tensor.transpose(
                                pt, xs[:, i, dk * P:(dk + 1) * P], ident)
                            nc.vector.tensor_copy(
                                xt[dk][:, ts * P:(ts + 1) * P], pt)

                # mm1 + gelu -> hT tiles (bf16, H on partitions)
                ht = [htp.tile([P, CHUNK], BF16, tag=f"ht{hm}",
                               name=f"ht{hm}_c{c}") for hm in range(HM)]
                for hm in range(HM):
                    p1 = p1p.tile([P, CHUNK], F32, tag="p1")
                    for dk in range(DK):
                        nc.tensor.matmul(
                            p1, w1_sb[:, dk, hm * P:(hm + 1) * P], xt[dk],
                            start=(dk == 0), stop=(dk == DK - 1))
                    nc.scalar.activation(ht[hm], p1, act,
                                         bias=b1_sb[:, hm:hm + 1], scale=1.0)

                # mm2 (+b2) -> out
                for ts in range(TSUB):
                    p2s = [p2p.tile([P, 512], F32, tag="p2",
                                    name=f"p2_c{c}t{ts}d{dc}")
                           for dc in range(DC)]
                    for hk in range(HM):
                        lhsT = ht[hk][:, ts * P:(ts + 1) * P]
                        for dc in range(DC):
                            nc.tensor.matmul(
                                p2s[dc], lhsT,
                                w2_sb[:, hk, dc * 512:(dc + 1) * 512],
                                start=(hk == 0), stop=(hk == HM - 1))
                    r0 = c * CHUNK + ts * P
                    for dc in range(DC):
                        ot = outp.tile([P, 512], F32, tag="ot")
                        nc.vector.tensor_add(
                            ot, p2s[dc], b2_rep[:, dc * 512:(dc + 1) * 512])
                        nc.sync.dma_start(
                            out[r0:r0 + P, dc * 512:(dc + 1) * 512], ot)

    nc.compile()
    return nc


_CACHE: dict = {}


def _program():
    if "nc" not in _CACHE:
        _CACHE["nc"] = build_program()
    return _CACHE["nc"]


def _in_maps(x, w1, b1, w2, b2):
    x = np.asarray(x, dtype=np.float32)
    w1 = np.asarray(w1, dtype=np.float32)
    b1 = np.asarray(b1, dtype=np.float32)
    w2 = np.asarray(w2, dtype=np.float32)
    b2 = np.asarray(b2, dtype=np.float32)
    maps = []
    for e in range(NCORES):
        maps.append({
            "x": np.ascontiguousarray(x[:, e].reshape(NTOK, D)),
            "w1": np.ascontiguousarray(w1[e]),
            "b1": np.ascontiguousarray(b1[e]),
            "w2": np.ascontiguousarray(w2[e]),
            "b2": np.ascontiguousarray(b2[e]),
        })
    return maps


def _install_ntff_hook_shim():
    """Provide antenv.axon_hooks if the image lacks it, wiring the NTFF
    profile hook straight to libaxon_pjrt.so (mirrors trn_agent_boot)."""
    import sys
    try:
        from antenv.axon_hooks import get_axon_ntff_profile_hook  # noqa: F401
        return
    except ImportError:
        pass
    import contextlib
    import ctypes
    import types

    import antenv

    hook = None
    so_path = "/opt/axon/libaxon_pjrt.so"
    try:
        lib = ctypes.CDLL(so_path)
        if hasattr(lib, "axon_start_nrt_profile"):
            lib.axon_start_nrt_profile.argtypes = [
                ctypes.POINTER(ctypes.c_int64), ctypes.c_size_t]
            lib.axon_start_nrt_profile.restype = ctypes.c_int64
            lib.axon_stop_nrt_profile.argtypes = [ctypes.c_char_p]
            lib.axon_stop_nrt_profile.restype = ctypes.c_int64

            @contextlib.contextmanager
            def _hook(output_dir, device_ids):
                import jax
                jax.devices()
                if device_ids:
                    ids = (ctypes.c_int64 * len(device_ids))(*device_ids)
                    rc = lib.axon_start_nrt_profile(ids, len(device_ids))
                else:
                    rc = lib.axon_start_nrt_profile(None, 0)
                if rc != 0:
                    raise RuntimeError(f"axon_start_nrt_profile rc={rc}")
                try:
                    yield
                finally:
                    n = lib.axon_stop_nrt_profile(str(output_dir).encode())
                    print(f"ntff profile: {n} file(s) -> {output_dir}")

            hook = _hook
    except OSError:
        pass

    mod = types.ModuleType("antenv.axon_hooks")
    mod._hook = hook
    mod.get_axon_ntff_profile_hook = lambda: mod._hook
    mod.set_axon_ntff_profile_hook = lambda h: setattr(mod, "_hook", h)
    sys.modules["antenv.axon_hooks"] = mod
    antenv.axon_hooks = mod


def run_spmd(x, w1, b1, w2, b2, trace=False):
    if trace:
        _install_ntff_hook_shim()
    nc = _program()
    res = bass_utils.run_bass_kernel_spmd(
        nc, _in_maps(x, w1, b1, w2, b2), core_ids=list(range(NCORES)),
        trace=trace)
    outs = [r["out"].reshape(4, 1024, D) for r in res.results]
    full = np.stack(outs, axis=1).astype(np.float32)  # [4, 8, 1024, 1024]
    return full, res


def kernel(x, w1, b1, w2, b2):
    full, _ = run_spmd(x, w1, b1, w2, b2)
    return full
